# revision 2
# baseline (speedup 1.0000x reference)
import sys
sys.path.insert(0, "/opt/trn_rl_repo")
import numpy as np
import concourse.bass as bass
from concourse import mybir
from concourse.bass_utils import run_bass_kernel_spmd

F32 = mybir.dt.float32
T, B, N = 100, 64, 2048
NCORES = 8
B_LOC = B // NCORES
BN = B_LOC * N
P = 128
FREE = BN // P
K = 10
C = T // K
KF = K * FREE


def _build_nc():
    nc = bass.Bass()
    x_ext = nc.dram_tensor("x", [T, BN], F32, kind="ExternalInput")
    r_ext = nc.dram_tensor("r", [T, BN], F32, kind="ExternalInput")
    s_ext = nc.dram_tensor("s", [T, BN], F32, kind="ExternalOutput")

    xv = x_ext.rearrange("t (p f) -> p t f", p=P, f=FREE)
    rv = r_ext.rearrange("t (p f) -> p t f", p=P, f=FREE)
    sv = s_ext.rearrange("t (p f) -> p t f", p=P, f=FREE)

    with (
        nc.sbuf_tensor([P, 2, KF], F32) as xb,
        nc.sbuf_tensor([P, 2, KF], F32) as rb,
        nc.sbuf_tensor([P, 2, KF], F32) as ub,
        nc.sbuf_tensor([P, 2, KF], F32) as vb,
        nc.sbuf_tensor([P, KF], F32) as sg,
        nc.sbuf_tensor([P, 2, KF], F32) as so,
        nc.sbuf_tensor([P, FREE], F32) as w,
        nc.sbuf_tensor([P, FREE], F32) as w0,
        nc.semaphore() as sem_in,
        nc.semaphore() as sem_dve,
        nc.semaphore() as sem_act,
        nc.semaphore() as sem_out,
        nc.Block() as block,
    ):
        @block.sync
        def _(sync):
            for c in range(C):
                b = c % 2
                if c >= 2:
                    sync.wait_ge(sem_dve, c - 1)
                sync.dma_start(xb[:, b, :], xv[:, c*K:(c+1)*K, :]).then_inc(sem_in, 16)
                sync.dma_start(rb[:, b, :], rv[:, c*K:(c+1)*K, :]).then_inc(sem_in, 16)
                if c >= 1:
                    sync.wait_ge(sem_act, c)
                    sync.dma_start(sv[:, (c-1)*K:c*K, :], so[:, (c-1) % 2, :]).then_inc(sem_out, 16)
            sync.wait_ge(sem_act, C)
            sync.dma_start(sv[:, (C-1)*K:C*K, :], so[:, (C-1) % 2, :]).then_inc(sem_out, 16)

        @block.vector
        def _(vector):
            nc.vector.memset(w0[:], 0.0)
            for c in range(C):
                b = c % 2
                if c >= 2:
                    vector.wait_ge(sem_act, c - 1)
                vector.wait_ge(sem_in, 32 * (c + 1))
                nc.vector.tensor_tensor(ub[:, b, :], xb[:, b, :], rb[:, b, :], mybir.AluOpType.add)
                for k in range(K):
                    t = c * K + k
                    if t == 0:
                        wsrc = w0
                    else:
                        vprev = (vb[:, (c-1) % 2, (K-1)*FREE:K*FREE] if k == 0
                                 else vb[:, b, (k-1)*FREE:k*FREE])
                        nc.vector.tensor_scalar(w[:], vprev, 0.0, 0.5,
                                                mybir.AluOpType.min, mybir.AluOpType.mult)
                        wsrc = w
                    ins = nc.vector.tensor_tensor(vb[:, b, k*FREE:(k+1)*FREE], wsrc[:],
                                                  ub[:, b, k*FREE:(k+1)*FREE], mybir.AluOpType.add)
                    if k == K - 1:
                        ins.then_inc(sem_dve, 1)

        @block.scalar
        def _(scalar):
            for c in range(C):
                b = c % 2
                if c >= 2:
                    scalar.wait_ge(sem_out, 16 * (c - 1))
                scalar.wait_ge(sem_dve, c + 1)
                nc.scalar.activation(sg[:], vb[:, b, :], mybir.ActivationFunctionType.Sign)
                nc.scalar.activation(so[:, b, :], sg[:], mybir.ActivationFunctionType.Relu).then_inc(sem_act, 1)

    return nc


def _make_in_maps(inp, rec):
    in_maps = []
    for i in range(NCORES):
        xs = np.ascontiguousarray(inp[:, i*B_LOC:(i+1)*B_LOC, :]).reshape(T, BN)
        rs = np.ascontiguousarray(rec[:, i*B_LOC:(i+1)*B_LOC, :]).reshape(T, BN)
        in_maps.append({"x": xs, "r": rs})
    return in_maps


def kernel(inp: np.ndarray, rec: np.ndarray) -> np.ndarray:
    inp = np.asarray(inp, dtype=np.float32)
    rec = np.asarray(rec, dtype=np.float32)
    nc = _build_nc()
    res = run_bass_kernel_spmd(nc, _make_in_maps(inp, rec), list(range(NCORES)))
    outs = [res.results[i]["s"].reshape(T, B_LOC, N) for i in range(NCORES)]
    return np.concatenate(outs, axis=1)


def run_traced(inp, rec, **kw):
    inp = np.asarray(inp, dtype=np.float32)
    rec = np.asarray(rec, dtype=np.float32)
    nc = _build_nc()
    return run_bass_kernel_spmd(nc, _make_in_maps(inp, rec),
                                list(range(NCORES)), trace=True, **kw)

